# revision 2
# baseline (speedup 1.0000x reference)
"""Trainium2 Bass kernel for a Keras SimpleRNN wrapper:

    xproj = inputs @ Wx + b                    # [B, T, H]
    h_t   = tanh(xproj_t + h_{t-1} @ Wh)       # scan over T, h_0 from zeros
    y     = h @ Wo + bo                        # [B, T, O]

Sharding: data-parallel over batch (dim 0) across 8 NeuronCores; the time
recurrence stays local per core. Weights are replicated.

Per-core layout is "H-major": the hidden state lives as h^T tiles
[128 partitions = H-within-chunk, free = 8*m_chunk + batch], so the PSUM
output layout of one recurrence step IS the stationary-operand layout of
the next step - no transposes anywhere.

Everything on-device computes in bf16 inputs with fp32 PSUM accumulation
(max abs error vs the fp32 reference is ~1.4e-2 on output scale ~3.4).
"""

import os
import numpy as np
import ml_dtypes

import concourse.bass as bass
import concourse.mybir as mybir
import concourse.tile as tile
from concourse import bacc
from concourse.bass import ds
from concourse.bass_utils import run_bass_kernel_spmd

BF16 = mybir.dt.bfloat16
FP32 = mybir.dt.float32
bfnp = ml_dtypes.bfloat16

B, T, D, H, O = 64, 512, 256, 1024, 128
NCORES = 8
BL = B // NCORES          # 8 batch rows per core
SLOT = 8 * BL             # 64: one h/xproj timestep slot = 8 m-chunks x 8 batch
KH = H // 128             # 8 contraction chunks for Wh/Wo
MH = H // 128             # 8 output chunks of H
KD = D // 128             # 2 contraction chunks for Wx
NTOK = BL * T             # 4096 tokens per core
TCHUNK = 64               # timesteps per hardware-loop iteration

_cached_nc = None

# Results of the last run (for the local test harness; unused when grading).
LAST_RESULTS = None


def _build():
    nc = bacc.Bacc("TRN2", target_bir_lowering=False, debug=False)

    xT = nc.dram_tensor("xT", [D, NTOK], BF16, kind="ExternalInput")
    wh = nc.dram_tensor("wh", [H, H], BF16, kind="ExternalInput")
    wx = nc.dram_tensor("wx", [D, H], BF16, kind="ExternalInput")
    wo = nc.dram_tensor("wo", [H, O], BF16, kind="ExternalInput")
    bv = nc.dram_tensor("bv", [H], FP32, kind="ExternalInput")
    bov = nc.dram_tensor("bov", [O], FP32, kind="ExternalInput")
    yT = nc.dram_tensor("yT", [O, NTOK], FP32, kind="ExternalOutput")

    with tile.TileContext(nc) as tc:
        with (
            tc.tile_pool(name="const", bufs=1) as const,
            tc.tile_pool(name="scr", bufs=2) as scr,
            tc.tile_pool(name="yout", bufs=2) as yout,
        ):
            # Persistent SBUF residents.
            XH = const.tile([128, T * SLOT], BF16)       # xproj, then h, per step
            WhS = const.tile([128, KH * H], BF16)        # Wh k-chunk k at [:, k*H:(k+1)*H]
            WxS = const.tile([128, KD * H], BF16)
            WoS = const.tile([128, KH * O], BF16)
            XTs = const.tile([128, KD * NTOK], BF16)
            bS = const.tile([128, MH], FP32)             # bS[p, m] = b[128m + p]
            boS = const.tile([128, 1], FP32)
            H0 = const.tile([128, SLOT], BF16)           # rotating h^T buffers
            H1 = const.tile([128, SLOT], BF16)

            for k in range(KH):
                nc.sync.dma_start(WhS[:, k * H:(k + 1) * H], wh[k * 128:(k + 1) * 128, :])
            for k in range(KD):
                nc.sync.dma_start(WxS[:, k * H:(k + 1) * H], wx[k * 128:(k + 1) * 128, :])
                nc.sync.dma_start(XTs[:, k * NTOK:(k + 1) * NTOK], xT[k * 128:(k + 1) * 128, :])
            for k in range(KH):
                nc.sync.dma_start(WoS[:, k * O:(k + 1) * O], wo[k * 128:(k + 1) * 128, :])
            nc.sync.dma_start(bS[:], bv[:].rearrange("(m p) -> p m", p=128))
            nc.sync.dma_start(boS[:], bov[:].rearrange("(p one) -> p one", one=1))
            nc.vector.memset(H0[:], 0.0)

            XH3 = XH[:].rearrange("p (s f) -> p s f", f=SLOT)  # [128, T, SLOT]

            # ---- Phase 1: xproj^T = Wx^T @ x^T + b, written into XH ----
            # Token tiles of 512 = 64 timesteps x 8 batch (t-major, b-minor).
            NT = 512
            with tc.tile_pool(name="p1psum", bufs=2, space="PSUM") as pp1:
                for nt in range(NTOK // NT):
                    for m in range(MH):
                        ps = pp1.tile([128, NT], FP32)
                        for k in range(KD):
                            nc.tensor.matmul(
                                ps[:],
                                WxS[:, k * H + 128 * m: k * H + 128 * (m + 1)],
                                XTs[:, k * NTOK + nt * NT: k * NTOK + (nt + 1) * NT],
                                start=(k == 0),
                                stop=(k == KD - 1),
                            )
                        dest = XH3[:, nt * 64:(nt + 1) * 64, 8 * m: 8 * (m + 1)]
                        nc.scalar.activation(
                            dest, ps[:],
                            mybir.ActivationFunctionType.Identity,
                            bias=bS[:, m:m + 1],
                        )

            # ---- Phase 2: the recurrence ----
            # Step s: PS[:, 8m:8m+8] = sum_k WhS_km^T @ Hprev[:, 8k:8k+8]
            #         z = PS + xproj_s (from XH);  Hcur = tanh(z);  XH_s = Hcur
            with tc.tile_pool(name="p2psum", bufs=2, space="PSUM") as pp2:
                def step(s_off, j):
                    hprev = H0 if j % 2 == 0 else H1
                    hcur = H1 if j % 2 == 0 else H0
                    ps = pp2.tile([128, SLOT], FP32, tag="ps2")
                    for m in range(MH):
                        for k in range(KH):
                            nc.tensor.matmul(
                                ps[:, 8 * m: 8 * (m + 1)],
                                WhS[:, k * H + 128 * m: k * H + 128 * (m + 1)],
                                hprev[:, 8 * k: 8 * (k + 1)],
                                start=(k == 0),
                                stop=(k == KH - 1),
                            )
                    z = scr.tile([128, SLOT], FP32, tag="z")
                    nc.vector.tensor_add(z[:], ps[:], XH[:, ds(s_off + j * SLOT, SLOT)])
                    nc.scalar.activation(hcur[:], z[:], mybir.ActivationFunctionType.Tanh)
                    nc.vector.tensor_copy(XH[:, ds(s_off + j * SLOT, SLOT)], hcur[:])

                with tc.For_i(0, T * SLOT, TCHUNK * SLOT) as s_off:
                    for j in range(TCHUNK):
                        step(s_off, j)

            # ---- Phase 3: y^T = Wo^T @ h^T + bo ----
            with tc.tile_pool(name="p3psum", bufs=2, space="PSUM") as pp3:
                for nt in range(NTOK // NT):
                    ps = pp3.tile([128, NT], FP32)
                    for k in range(KH):
                        nc.tensor.matmul(
                            ps[:],
                            WoS[:, k * O:(k + 1) * O],
                            XH3[:, nt * 64:(nt + 1) * 64, 8 * k: 8 * (k + 1)],
                            start=(k == 0),
                            stop=(k == KH - 1),
                        )
                    yt = yout.tile([128, NT], FP32, tag="yt")
                    nc.scalar.activation(
                        yt[:], ps[:],
                        mybir.ActivationFunctionType.Identity,
                        bias=boS[:],
                    )
                    nc.sync.dma_start(yT[:, nt * NT:(nt + 1) * NT], yt[:])

    nc.compile()
    return nc


def _get_nc():
    global _cached_nc
    if _cached_nc is None:
        _cached_nc = _build()
    return _cached_nc


def kernel(inputs, Wx, Wh, b, Wo, bo):
    global LAST_RESULTS
    x = np.asarray(inputs, dtype=np.float32)        # [B, T, D]
    nc = _get_nc()

    xT_full = np.ascontiguousarray(x.transpose(2, 1, 0)).astype(bfnp)  # [D, T, B]
    whb = np.asarray(Wh, np.float32).astype(bfnp)
    wxb = np.asarray(Wx, np.float32).astype(bfnp)
    wob = np.asarray(Wo, np.float32).astype(bfnp)
    bf = np.ascontiguousarray(np.asarray(b, np.float32))
    bof = np.ascontiguousarray(np.asarray(bo, np.float32))

    in_maps = []
    for c in range(NCORES):
        xs = np.ascontiguousarray(xT_full[:, :, c * BL:(c + 1) * BL]).reshape(D, NTOK)
        in_maps.append({
            "xT": xs, "wh": whb, "wx": wxb, "wo": wob, "bv": bf, "bov": bof,
        })

    res = run_bass_kernel_spmd(nc, in_maps, list(range(NCORES)))
    LAST_RESULTS = res

    y = np.empty((B, T, O), np.float32)
    for c in range(NCORES):
        ytc = res.results[c]["yT"]                   # [O, T*BL], col = t*BL + b
        y[c * BL:(c + 1) * BL] = ytc.reshape(O, T, BL).transpose(2, 1, 0)
    return y
